# revision 3
# baseline (speedup 1.0000x reference)
"""Trainium2 Bass kernel for nn_GRU (dual GRU-cell generator, h=0 variant).

Math per step (chain c in {d, p}, state code c_t):
  gi   = c_t @ Wih.T                        [B, 3H]
  r    = sigmoid(gi_r + b_r)                b_r  = bih_r + bhh_r
  1-z  = sigmoid(-(gi_z + b_z))             b_z  = bih_z + bhh_z
  n    = tanh(gi_n + b_in + r * b_hn)       b_in = bih_n, b_hn = bhh_n
  h    = (1-z) * n                          [B, H]
  c_t1 = sigmoid(h @ Wout.T + b_out)        [B, D]

Sharding: data-parallel over batch across 8 cores (B=512 -> 64/core).
The sequential scan stays local per core; weights are replicated and
streamed from HBM each step (bf16).

Device layout is feature-major ("transposed"): activations live as
[feat_part, ktile, batch] so matmul rhs slices are [128, 64] and the
ACT per-partition bias applies along the feature dim.  All outputs are
written bf16 feature-major; the host transposes/casts back.
"""

import sys

sys.path.insert(0, "/opt/trn_rl_repo")

import numpy as np
import ml_dtypes

BF16 = ml_dtypes.bfloat16

B, H, D, P, T = 512, 1024, 4096, 2048, 48
NCORES = 8
BC = B // NCORES  # 64 batch per core
KD, KP, KH = D // 128, P // 128, H // 128  # 32, 16, 8
G3 = 3 * H // 128  # 24 m-tiles for gi
DT, PT = D // 128, P // 128  # readout m-tiles: 32, 16

_compiled = {}


def _build(T_steps):
    """Build the SPMD bass program (same graph on all 8 cores)."""
    import concourse.bass as bass
    import concourse.bacc as bacc
    import concourse.tile as tile
    import concourse.mybir as mybir

    f32 = mybir.dt.float32
    bf16 = mybir.dt.bfloat16
    AF = mybir.ActivationFunctionType
    ALU = mybir.AluOpType

    nc = bacc.Bacc("TRN2", target_bir_lowering=False, debug=False,
                   num_devices=NCORES)

    # ---- DRAM I/O ----
    noiseT = nc.dram_tensor("noiseT", [128, KH, BC], bf16, kind="ExternalInput")
    dwih = nc.dram_tensor("dwih", [128, KD, 3 * H], bf16, kind="ExternalInput")
    pwih = nc.dram_tensor("pwih", [128, KP, 3 * H], bf16, kind="ExternalInput")
    dwout = nc.dram_tensor("dwout", [128, KH, D], bf16, kind="ExternalInput")
    pwout = nc.dram_tensor("pwout", [128, KH, P], bf16, kind="ExternalInput")
    dgb = nc.dram_tensor("dgb", [128, KH, 4], f32, kind="ExternalInput")
    pgb = nc.dram_tensor("pgb", [128, KH, 4], f32, kind="ExternalInput")
    dob = nc.dram_tensor("dob", [128, DT], f32, kind="ExternalInput")
    pob = nc.dram_tensor("pob", [128, PT], f32, kind="ExternalInput")

    ds_o = nc.dram_tensor("ds", [128, DT, T_steps + 1, BC], bf16, kind="ExternalOutput")
    ps_o = nc.dram_tensor("ps", [128, PT, T_steps + 1, BC], bf16, kind="ExternalOutput")
    dh_o = nc.dram_tensor("dh", [128, KH, T_steps, BC], bf16, kind="ExternalOutput")
    ph_o = nc.dram_tensor("ph", [128, KH, T_steps, BC], bf16, kind="ExternalOutput")

    with tile.TileContext(nc) as tc:
        from contextlib import ExitStack

        ctx = ExitStack()
        with ctx:
            # SBUF pools
            wdih_p = ctx.enter_context(tc.tile_pool(name="wdih", bufs=3))
            wpih_p = ctx.enter_context(tc.tile_pool(name="wpih", bufs=2))
            wdout_p = ctx.enter_context(tc.tile_pool(name="wdout", bufs=2))
            wpout_p = ctx.enter_context(tc.tile_pool(name="wpout", bufs=2))
            state_p = ctx.enter_context(tc.tile_pool(name="state", bufs=3))
            hbuf_p = ctx.enter_context(tc.tile_pool(name="hbuf", bufs=2))
            gt_p = ctx.enter_context(tc.tile_pool(name="gt", bufs=12))
            const_p = ctx.enter_context(tc.tile_pool(name="const", bufs=1))
            # PSUM pools
            gi_p = ctx.enter_context(tc.tile_pool(name="gi", bufs=3, space="PSUM"))
            ro_p = ctx.enter_context(tc.tile_pool(name="ro", bufs=4, space="PSUM"))

            # ---- constants ----
            dgb_sb = const_p.tile([128, KH, 4], f32, tag="dgb")
            pgb_sb = const_p.tile([128, KH, 4], f32, tag="pgb")
            dob_sb = const_p.tile([128, DT], f32, tag="dob")
            pob_sb = const_p.tile([128, PT], f32, tag="pob")
            noise_sb = const_p.tile([128, KH, BC], bf16, tag="noise")
            nc.sync.dma_start(out=dgb_sb[:], in_=dgb[:])
            nc.sync.dma_start(out=pgb_sb[:], in_=pgb[:])
            nc.sync.dma_start(out=dob_sb[:], in_=dob[:])
            nc.sync.dma_start(out=pob_sb[:], in_=pob[:])
            nc.sync.dma_start(out=noise_sb[:], in_=noiseT[:])

            def readout(rhs_tile, w_dram, w_pool, w_cols, mt, bias_sb, out_code,
                        out_dram, slot):
                """code = sigmoid(Wout @ h + b); also DMA to out_dram[:,:,slot,:].

                rhs_tile: [128, KH, BC] bf16 (h, feature-major)
                w_dram:   [KH, 128, w_cols] bf16
                out_code: [128, mt, BC] bf16 tile (written)
                """
                ngroups = (mt + 7) // 8
                ro_tiles = [ro_p.tile([128, 512], f32, tag="ro", name=f"rot{i}") for i in range(ngroups)]
                for s in range(KH // 2):
                    wt = w_pool.tile([128, 2, w_cols], bf16)
                    nc.sync.dma_start(out=wt[:], in_=w_dram[:, 2 * s:2 * s + 2, :])
                    for j in range(2):
                        k = 2 * s + j
                        for m in range(mt):
                            g, mm = divmod(m, 8)
                            nc.tensor.matmul(
                                ro_tiles[g][:, mm * BC:(mm + 1) * BC],
                                wt[:, j, m * 128:(m + 1) * 128],
                                rhs_tile[:, k, :],
                                start=(k == 0 and mm == 0),
                                stop=(k == KH - 1 and m == mt - 1),
                                skip_group_check=True,
                            )
                for m in range(mt):
                    g, mm = divmod(m, 8)
                    nc.scalar.activation(
                        out_code[:, m, :],
                        ro_tiles[g][:, mm * BC:(mm + 1) * BC],
                        AF.Sigmoid,
                        bias=bias_sb[:, m:m + 1],
                    )
                nc.gpsimd.dma_start(out=out_dram[:, :, slot, :], in_=out_code[:])

            def gru_half(code_tile, kt, w_dram, w_pool, gb_sb, h_out):
                """h = GRU_zero_h(code); h_out [128, KH, BC] bf16."""
                gi_tiles = [gi_p.tile([128, 512], f32, tag="gi", name=f"git{i}") for i in range(3)]
                for s in range(kt // 2):
                    wt = w_pool.tile([128, 2, 3 * H], bf16)
                    nc.sync.dma_start(out=wt[:], in_=w_dram[:, 2 * s:2 * s + 2, :])
                    for j in range(2):
                        k = 2 * s + j
                        for m in range(G3):
                            g, mm = divmod(m, 8)
                            nc.tensor.matmul(
                                gi_tiles[g][:, mm * BC:(mm + 1) * BC],
                                wt[:, j, m * 128:(m + 1) * 128],
                                code_tile[:, k, :],
                                start=(k == 0 and mm == 0),
                                stop=(k == kt - 1 and m == G3 - 1),
                                skip_group_check=True,
                            )
                # gates, tile by tile over H
                for jt in range(KH):
                    sl = slice(jt * BC, (jt + 1) * BC)
                    i_r = gi_tiles[0][:, sl]
                    i_z = gi_tiles[1][:, sl]
                    i_n = gi_tiles[2][:, sl]
                    r = gt_p.tile([128, BC], f32, tag="r")
                    zm = gt_p.tile([128, BC], f32, tag="zm")
                    rb = gt_p.tile([128, BC], f32, tag="rb")
                    nin = gt_p.tile([128, BC], f32, tag="nin")
                    nt = gt_p.tile([128, BC], f32, tag="nt")
                    # r = sigmoid(i_r + b_r)
                    nc.scalar.activation(r[:], i_r, AF.Sigmoid,
                                         bias=gb_sb[:, jt, 0:1])
                    # 1-z = sigmoid(-i_z + (-b_z))   (col 1 stores -b_z)
                    nc.scalar.activation(zm[:], i_z, AF.Sigmoid,
                                         bias=gb_sb[:, jt, 1:2], scale=-1.0)
                    # rb = r * b_hn
                    nc.vector.tensor_scalar(rb[:], r[:], gb_sb[:, jt, 2:3], None,
                                            ALU.mult)
                    # nin = i_n + rb
                    nc.vector.tensor_tensor(nin[:], i_n, rb[:], ALU.add)
                    # n = tanh(nin + b_in)
                    nc.scalar.activation(nt[:], nin[:], AF.Tanh,
                                         bias=gb_sb[:, jt, 3:4])
                    # h = zm * n  (cast to bf16 on write)
                    nc.vector.tensor_tensor(h_out[:, jt, :], zm[:], nt[:],
                                            ALU.mult)

            # ---- init: dc0 = sigmoid(Wout @ noiseT + b) ----
            dc = state_p.tile([128, KD, BC], bf16, tag="dc")
            readout(noise_sb, dwout, wdout_p, D, DT, dob_sb, dc, ds_o, 0)
            pc = state_p.tile([128, KP, BC], bf16, tag="pc")
            readout(noise_sb, pwout, wpout_p, P, PT, pob_sb, pc, ps_o, 0)

            # ---- the scan ----
            for t in range(T_steps):
                hd = hbuf_p.tile([128, KH, BC], bf16, tag="hd")
                gru_half(dc, KD, dwih, wdih_p, dgb_sb, hd)
                nc.gpsimd.dma_start(out=dh_o[:, :, t, :], in_=hd[:])

                hp = hbuf_p.tile([128, KH, BC], bf16, tag="hp")
                gru_half(pc, KP, pwih, wpih_p, pgb_sb, hp)
                nc.gpsimd.dma_start(out=ph_o[:, :, t, :], in_=hp[:])

                dc = state_p.tile([128, KD, BC], bf16, tag="dc")
                readout(hd, dwout, wdout_p, D, DT, dob_sb, dc, ds_o, t + 1)
                pc = state_p.tile([128, KP, BC], bf16, tag="pc")
                readout(hp, pwout, wpout_p, P, PT, pob_sb, pc, ps_o, t + 1)

    nc.compile()
    return nc


def _get_nc(T_steps):
    if T_steps not in _compiled:
        _compiled[T_steps] = _build(T_steps)
    return _compiled[T_steps]


def _prep_inputs(noise, d_Wih, d_bih, d_bhh, p_Wih, p_bih, p_bhh,
                 d_Wout, d_bout, p_Wout, p_bout):
    """Host-side packing: transposes, bf16 casts, bias folding."""
    def wt(w, kt):  # [out, in] -> [128, kt, out] bf16
        return np.ascontiguousarray(
            w.T.reshape(kt, 128, w.shape[0]).transpose(1, 0, 2)).astype(BF16)

    dwih = wt(d_Wih, KD)
    pwih = wt(p_Wih, KP)
    dwout = wt(d_Wout, KH)
    pwout = wt(p_Wout, KH)

    def gate_bias(bih, bhh):
        b_r = (bih[:H] + bhh[:H]).reshape(KH, 128).T
        b_zn = -(bih[H:2 * H] + bhh[H:2 * H]).reshape(KH, 128).T
        b_hn = bhh[2 * H:].reshape(KH, 128).T
        b_in = bih[2 * H:].reshape(KH, 128).T
        return np.ascontiguousarray(
            np.stack([b_r, b_zn, b_hn, b_in], axis=2)).astype(np.float32)

    dgb = gate_bias(d_bih, d_bhh)
    pgb = gate_bias(p_bih, p_bhh)
    dob = np.ascontiguousarray(d_bout.reshape(DT, 128).T).astype(np.float32)
    pob = np.ascontiguousarray(p_bout.reshape(PT, 128).T).astype(np.float32)

    in_maps = []
    for c in range(NCORES):
        nT = np.ascontiguousarray(
            noise[c * BC:(c + 1) * BC].T.reshape(KH, 128, BC)
            .transpose(1, 0, 2)).astype(BF16)
        in_maps.append({
            "noiseT": nT, "dwih": dwih, "pwih": pwih,
            "dwout": dwout, "pwout": pwout,
            "dgb": dgb, "pgb": pgb, "dob": dob, "pob": pob,
        })
    return in_maps


def _unshard(results, T_steps):
    """Device feature-major bf16 -> full batch-major fp32."""
    def collect(name, mt, tslice):
        # per-core [128, mt, T?, BC] -> [BC, T, mt*128]
        parts = []
        for c in range(NCORES):
            a = np.asarray(results[c][name])[:, :, tslice, :]
            # [p, m, t, b] -> [b, t, m, p] -> [b, t, F]
            a = a.transpose(3, 2, 1, 0).reshape(BC, T_steps, mt * 128)
            parts.append(a)
        return np.concatenate(parts, axis=0).astype(np.float32)

    ds = collect("ds", DT, slice(0, T_steps))
    ps = collect("ps", PT, slice(0, T_steps))
    dh = collect("dh", KH, slice(0, T_steps))
    ph = collect("ph", KH, slice(0, T_steps))
    return (ds, ps), (dh, ph)


def kernel(noise, d_Wih, d_Whh, d_bih, d_bhh, p_Wih, p_Whh, p_bih, p_bhh,
           d_Wout, d_bout, p_Wout, p_bout, max_len, _trace=False):
    T_steps = int(max_len)
    f = np.asarray
    in_maps = _prep_inputs(f(noise), f(d_Wih), f(d_bih), f(d_bhh),
                           f(p_Wih), f(p_bih), f(p_bhh),
                           f(d_Wout), f(d_bout), f(p_Wout), f(p_bout))
    nc = _get_nc(T_steps)
    from concourse.bass_utils import run_bass_kernel_spmd
    res = run_bass_kernel_spmd(nc, in_maps, list(range(NCORES)), trace=_trace)
    out = _unshard(res.results, T_steps)
    if _trace:
        return out, res
    return out


# revision 4
# speedup vs baseline: 1.2680x; 1.2680x over previous
"""Trainium2 Bass kernel for nn_GRU (dual GRU-cell generator, h=0 variant).

Math per step (chain c in {d, p}, state code c_t):
  gi   = c_t @ Wih.T                        [B, 3H]
  r    = sigmoid(gi_r + b_r)                b_r  = bih_r + bhh_r
  1-z  = sigmoid(-(gi_z + b_z))             b_z  = bih_z + bhh_z
  n    = tanh(gi_n + b_in + r * b_hn)       b_in = bih_n, b_hn = bhh_n
  h    = (1-z) * n                          [B, H]
  c_t1 = sigmoid(h @ Wout.T + b_out)        [B, D]

Sharding: data-parallel over batch across 8 cores (B=512 -> 64/core).
The sequential scan stays local per core; weights are replicated and
streamed from HBM each step (bf16).

Device layout is feature-major ("transposed"): activations live as
[feat_part, ktile, batch] so matmul rhs slices are [128, 64] and the
ACT per-partition bias applies along the feature dim.  All outputs are
written bf16 feature-major; the host transposes/casts back.
"""

import sys

sys.path.insert(0, "/opt/trn_rl_repo")

import numpy as np
import ml_dtypes

BF16 = ml_dtypes.bfloat16

B, H, D, P, T = 512, 1024, 4096, 2048, 48
NCORES = 8
BC = B // NCORES  # 64 batch per core
KD, KP, KH = D // 128, P // 128, H // 128  # 32, 16, 8
G3 = 3 * H // 128  # 24 m-tiles for gi
DT, PT = D // 128, P // 128  # readout m-tiles: 32, 16

_compiled = {}


def _build(T_steps):
    """Build the SPMD bass program (same graph on all 8 cores)."""
    import concourse.bass as bass
    import concourse.bacc as bacc
    import concourse.tile as tile
    import concourse.mybir as mybir

    f32 = mybir.dt.float32
    bf16 = mybir.dt.bfloat16
    AF = mybir.ActivationFunctionType
    ALU = mybir.AluOpType

    nc = bacc.Bacc("TRN2", target_bir_lowering=False, debug=False,
                   num_devices=NCORES)

    # ---- DRAM I/O ----
    noiseT = nc.dram_tensor("noiseT", [128, KH, BC], bf16, kind="ExternalInput")
    dwih = nc.dram_tensor("dwih", [128, KD, 3 * H], bf16, kind="ExternalInput")
    pwih = nc.dram_tensor("pwih", [128, KP, 3 * H], bf16, kind="ExternalInput")
    dwout = nc.dram_tensor("dwout", [128, KH, D], bf16, kind="ExternalInput")
    pwout = nc.dram_tensor("pwout", [128, KH, P], bf16, kind="ExternalInput")
    dgb = nc.dram_tensor("dgb", [128, KH, 4], f32, kind="ExternalInput")
    pgb = nc.dram_tensor("pgb", [128, KH, 4], f32, kind="ExternalInput")
    dob = nc.dram_tensor("dob", [128, DT], f32, kind="ExternalInput")
    pob = nc.dram_tensor("pob", [128, PT], f32, kind="ExternalInput")

    ds_o = nc.dram_tensor("ds", [128, DT, T_steps + 1, BC], bf16, kind="ExternalOutput")
    ps_o = nc.dram_tensor("ps", [128, PT, T_steps + 1, BC], bf16, kind="ExternalOutput")
    dh_o = nc.dram_tensor("dh", [128, KH, T_steps, BC], bf16, kind="ExternalOutput")
    ph_o = nc.dram_tensor("ph", [128, KH, T_steps, BC], bf16, kind="ExternalOutput")

    with tile.TileContext(nc) as tc:
        from contextlib import ExitStack

        ctx = ExitStack()
        with ctx:
            # SBUF pools
            wdih_p = ctx.enter_context(tc.tile_pool(name="wdih", bufs=3))
            wpih_p = ctx.enter_context(tc.tile_pool(name="wpih", bufs=2))
            state_p = ctx.enter_context(tc.tile_pool(name="state", bufs=3))
            hbuf_p = ctx.enter_context(tc.tile_pool(name="hbuf", bufs=2))
            gt_p = ctx.enter_context(tc.tile_pool(name="gt", bufs=12))
            const_p = ctx.enter_context(tc.tile_pool(name="const", bufs=1))
            # PSUM pools
            gi_p = ctx.enter_context(tc.tile_pool(name="gi", bufs=3, space="PSUM"))
            ro_p = ctx.enter_context(tc.tile_pool(name="ro", bufs=4, space="PSUM"))

            # ---- constants ----
            dgb_sb = const_p.tile([128, KH, 4], f32, tag="dgb")
            pgb_sb = const_p.tile([128, KH, 4], f32, tag="pgb")
            dob_sb = const_p.tile([128, DT], f32, tag="dob")
            pob_sb = const_p.tile([128, PT], f32, tag="pob")
            noise_sb = const_p.tile([128, KH, BC], bf16, tag="noise")
            dwout_sb = const_p.tile([128, KH, D], bf16, tag="dwout_sb")
            pwout_sb = const_p.tile([128, KH, P], bf16, tag="pwout_sb")
            nc.sync.dma_start(out=dwout_sb[:], in_=dwout[:])
            nc.sync.dma_start(out=pwout_sb[:], in_=pwout[:])
            nc.sync.dma_start(out=dgb_sb[:], in_=dgb[:])
            nc.sync.dma_start(out=pgb_sb[:], in_=pgb[:])
            nc.sync.dma_start(out=dob_sb[:], in_=dob[:])
            nc.sync.dma_start(out=pob_sb[:], in_=pob[:])
            nc.sync.dma_start(out=noise_sb[:], in_=noiseT[:])

            def readout(rhs_tile, w_sb, w_cols, mt, bias_sb, out_code,
                        out_dram, slot):
                """code = sigmoid(Wout @ h + b); also DMA to out_dram[:,:,slot,:].

                rhs_tile: [128, KH, BC] bf16 (h, feature-major)
                w_sb:     [128, KH, w_cols] bf16 resident SBUF tile
                out_code: [128, mt, BC] bf16 tile (written)
                """
                ngroups = (mt + 7) // 8
                ro_tiles = [ro_p.tile([128, 512], f32, tag="ro", name=f"rot{i}") for i in range(ngroups)]
                for k in range(KH):
                    for m in range(mt):
                        g, mm = divmod(m, 8)
                        nc.tensor.matmul(
                            ro_tiles[g][:, mm * BC:(mm + 1) * BC],
                            w_sb[:, k, m * 128:(m + 1) * 128],
                            rhs_tile[:, k, :],
                            start=(k == 0 and mm == 0),
                            stop=(k == KH - 1 and m == mt - 1),
                            skip_group_check=True,
                        )
                for m in range(mt):
                    g, mm = divmod(m, 8)
                    nc.scalar.activation(
                        out_code[:, m, :],
                        ro_tiles[g][:, mm * BC:(mm + 1) * BC],
                        AF.Sigmoid,
                        bias=bias_sb[:, m:m + 1],
                    )
                nc.gpsimd.dma_start(out=out_dram[:, :, slot, :], in_=out_code[:])

            def gru_half(code_tile, kt, w_dram, w_pool, gb_sb, h_out):
                """h = GRU_zero_h(code); h_out [128, KH, BC] bf16."""
                gi_tiles = [gi_p.tile([128, 512], f32, tag="gi", name=f"git{i}") for i in range(3)]
                for s in range(kt // 2):
                    wt = w_pool.tile([128, 2, 3 * H], bf16)
                    nc.sync.dma_start(out=wt[:], in_=w_dram[:, 2 * s:2 * s + 2, :])
                    for j in range(2):
                        k = 2 * s + j
                        for m in range(G3):
                            g, mm = divmod(m, 8)
                            nc.tensor.matmul(
                                gi_tiles[g][:, mm * BC:(mm + 1) * BC],
                                wt[:, j, m * 128:(m + 1) * 128],
                                code_tile[:, k, :],
                                start=(k == 0 and mm == 0),
                                stop=(k == kt - 1 and m == G3 - 1),
                                skip_group_check=True,
                            )
                # gates, tile by tile over H
                for jt in range(KH):
                    sl = slice(jt * BC, (jt + 1) * BC)
                    i_r = gi_tiles[0][:, sl]
                    i_z = gi_tiles[1][:, sl]
                    i_n = gi_tiles[2][:, sl]
                    r = gt_p.tile([128, BC], f32, tag="r")
                    zm = gt_p.tile([128, BC], f32, tag="zm")
                    rb = gt_p.tile([128, BC], f32, tag="rb")
                    nin = gt_p.tile([128, BC], f32, tag="nin")
                    nt = gt_p.tile([128, BC], f32, tag="nt")
                    # r = sigmoid(i_r + b_r)
                    nc.scalar.activation(r[:], i_r, AF.Sigmoid,
                                         bias=gb_sb[:, jt, 0:1])
                    # 1-z = sigmoid(-i_z + (-b_z))   (col 1 stores -b_z)
                    nc.scalar.activation(zm[:], i_z, AF.Sigmoid,
                                         bias=gb_sb[:, jt, 1:2], scale=-1.0)
                    # rb = r * b_hn
                    nc.vector.tensor_scalar(rb[:], r[:], gb_sb[:, jt, 2:3], None,
                                            ALU.mult)
                    # nin = i_n + rb
                    nc.vector.tensor_tensor(nin[:], i_n, rb[:], ALU.add)
                    # n = tanh(nin + b_in)
                    nc.scalar.activation(nt[:], nin[:], AF.Tanh,
                                         bias=gb_sb[:, jt, 3:4])
                    # h = zm * n  (cast to bf16 on write)
                    nc.vector.tensor_tensor(h_out[:, jt, :], zm[:], nt[:],
                                            ALU.mult)

            # ---- init: dc0 = sigmoid(Wout @ noiseT + b) ----
            dc = state_p.tile([128, KD, BC], bf16, tag="dc")
            readout(noise_sb, dwout_sb, D, DT, dob_sb, dc, ds_o, 0)
            pc = state_p.tile([128, KP, BC], bf16, tag="pc")
            readout(noise_sb, pwout_sb, P, PT, pob_sb, pc, ps_o, 0)

            # ---- the scan ----
            for t in range(T_steps):
                hd = hbuf_p.tile([128, KH, BC], bf16, tag="hd")
                gru_half(dc, KD, dwih, wdih_p, dgb_sb, hd)
                nc.gpsimd.dma_start(out=dh_o[:, :, t, :], in_=hd[:])

                hp = hbuf_p.tile([128, KH, BC], bf16, tag="hp")
                gru_half(pc, KP, pwih, wpih_p, pgb_sb, hp)
                nc.gpsimd.dma_start(out=ph_o[:, :, t, :], in_=hp[:])

                dc = state_p.tile([128, KD, BC], bf16, tag="dc")
                readout(hd, dwout_sb, D, DT, dob_sb, dc, ds_o, t + 1)
                pc = state_p.tile([128, KP, BC], bf16, tag="pc")
                readout(hp, pwout_sb, P, PT, pob_sb, pc, ps_o, t + 1)

    nc.compile()
    return nc


def _get_nc(T_steps):
    if T_steps not in _compiled:
        _compiled[T_steps] = _build(T_steps)
    return _compiled[T_steps]


def _prep_inputs(noise, d_Wih, d_bih, d_bhh, p_Wih, p_bih, p_bhh,
                 d_Wout, d_bout, p_Wout, p_bout):
    """Host-side packing: transposes, bf16 casts, bias folding."""
    def wt(w, kt):  # [out, in] -> [128, kt, out] bf16
        return np.ascontiguousarray(
            w.T.reshape(kt, 128, w.shape[0]).transpose(1, 0, 2)).astype(BF16)

    dwih = wt(d_Wih, KD)
    pwih = wt(p_Wih, KP)
    dwout = wt(d_Wout, KH)
    pwout = wt(p_Wout, KH)

    def gate_bias(bih, bhh):
        b_r = (bih[:H] + bhh[:H]).reshape(KH, 128).T
        b_zn = -(bih[H:2 * H] + bhh[H:2 * H]).reshape(KH, 128).T
        b_hn = bhh[2 * H:].reshape(KH, 128).T
        b_in = bih[2 * H:].reshape(KH, 128).T
        return np.ascontiguousarray(
            np.stack([b_r, b_zn, b_hn, b_in], axis=2)).astype(np.float32)

    dgb = gate_bias(d_bih, d_bhh)
    pgb = gate_bias(p_bih, p_bhh)
    dob = np.ascontiguousarray(d_bout.reshape(DT, 128).T).astype(np.float32)
    pob = np.ascontiguousarray(p_bout.reshape(PT, 128).T).astype(np.float32)

    in_maps = []
    for c in range(NCORES):
        nT = np.ascontiguousarray(
            noise[c * BC:(c + 1) * BC].T.reshape(KH, 128, BC)
            .transpose(1, 0, 2)).astype(BF16)
        in_maps.append({
            "noiseT": nT, "dwih": dwih, "pwih": pwih,
            "dwout": dwout, "pwout": pwout,
            "dgb": dgb, "pgb": pgb, "dob": dob, "pob": pob,
        })
    return in_maps


def _unshard(results, T_steps):
    """Device feature-major bf16 -> full batch-major fp32."""
    def collect(name, mt, tslice):
        # per-core [128, mt, T?, BC] -> [BC, T, mt*128]
        parts = []
        for c in range(NCORES):
            a = np.asarray(results[c][name])[:, :, tslice, :]
            # [p, m, t, b] -> [b, t, m, p] -> [b, t, F]
            a = a.transpose(3, 2, 1, 0).reshape(BC, T_steps, mt * 128)
            parts.append(a)
        return np.concatenate(parts, axis=0).astype(np.float32)

    ds = collect("ds", DT, slice(0, T_steps))
    ps = collect("ps", PT, slice(0, T_steps))
    dh = collect("dh", KH, slice(0, T_steps))
    ph = collect("ph", KH, slice(0, T_steps))
    return (ds, ps), (dh, ph)


def kernel(noise, d_Wih, d_Whh, d_bih, d_bhh, p_Wih, p_Whh, p_bih, p_bhh,
           d_Wout, d_bout, p_Wout, p_bout, max_len, _trace=False):
    T_steps = int(max_len)
    f = np.asarray
    in_maps = _prep_inputs(f(noise), f(d_Wih), f(d_bih), f(d_bhh),
                           f(p_Wih), f(p_bih), f(p_bhh),
                           f(d_Wout), f(d_bout), f(p_Wout), f(p_bout))
    nc = _get_nc(T_steps)
    from concourse.bass_utils import run_bass_kernel_spmd
    res = run_bass_kernel_spmd(nc, in_maps, list(range(NCORES)), trace=_trace)
    out = _unshard(res.results, T_steps)
    if _trace:
        return out, res
    return out
